# revision 1
# baseline (speedup 1.0000x reference)
"""Trainium2 Bass kernel for nn_Adapter_CrossNonParam (adapter + prompt/token cross-attention).

Data-parallel over batch: 8 NeuronCores x 4 batches each. Adapter weights are
replicated. All matmuls run in bf16 (fp32 PSUM accumulation); input x is
pre-transposed and cast to bf16 on the host so the device kernel needs no
layout shuffles for the big tensors.

Per-batch device pipeline (everything kept D-on-partition):
  downT[D,N] = W_down^T @ xT            (8 C-tile accumulation, PSUM)
  downT = gelu(downT + b_down)          (ScalarE, erf gelu, fused PSUM->SBUF, bf16)
  logitsT[t,P] = token_downT_t^T @ prompt_downT     (16 tiles)
  expT = exp(SCALE * logitsT)           (no max subtraction; logits are O(5))
  denom[1,P] = ones^T @ expT            (accumulated ones-matmul)
  prompt_outT[D,P] += tok_tr_t^T @ expT_t   (tok_tr = PE-transposed token tiles)
  prompt_outT *= bcast(1/denom)         (PE broadcast + VectorE multiply)
  combT = [prompt_outT | token_downT]   (in-place in the downT buffer)
  up[n,C] = combT_n^T @ W_up ; out = up + b_up   (bias fused into PSUM->SBUF copy)
"""
import numpy as np
import ml_dtypes

import concourse.bass as bass
import concourse.tile as tile
from concourse import bacc, mybir
from concourse.bass_utils import run_bass_kernel_spmd

BF = mybir.dt.bfloat16
F32 = mybir.dt.float32

B, N, C = 32, 2248, 1024
D = 128
P = 200
T = N - P  # 2048
NCORES = 8
NB = B // NCORES  # 4 batches per core
SCALE = float(D) ** -0.5

CTILES = C // 128  # 8
TTILES = T // 128  # 16
DOWN_CHUNKS = [(s, min(512, N - s)) for s in range(0, N, 512)]  # 4x512 + 200
UP_TILES = [(s, min(128, N - s)) for s in range(0, N, 128)]  # 17x128 + 72


def build_nc():
    nc = bacc.Bacc("TRN2", target_bir_lowering=False, debug=False, num_devices=NCORES)

    xT = nc.dram_tensor("xT", [NB, C, N], BF, kind="ExternalInput")
    wdn = nc.dram_tensor("wdn", [128, CTILES, 128], BF, kind="ExternalInput")
    wup = nc.dram_tensor("wup", [D, C], BF, kind="ExternalInput")
    bdn = nc.dram_tensor("bdn", [D, 1], F32, kind="ExternalInput")
    bup = nc.dram_tensor("bup", [128, C], F32, kind="ExternalInput")
    ident = nc.dram_tensor("ident", [128, 128], BF, kind="ExternalInput")
    onesb = nc.dram_tensor("onesb", [128, 1], BF, kind="ExternalInput")
    ones1 = nc.dram_tensor("ones1", [1, 128], F32, kind="ExternalInput")
    out = nc.dram_tensor("out", [NB, N, C], F32, kind="ExternalOutput")

    with tile.TileContext(nc) as tc:
        with (
            tc.tile_pool(name="const", bufs=1) as const,
            tc.tile_pool(name="xp", bufs=2) as xp,
            tc.tile_pool(name="dg", bufs=2) as dg,
            tc.tile_pool(name="ex", bufs=2) as ex,
            tc.tile_pool(name="tt", bufs=2) as tt,
            tc.tile_pool(name="ob", bufs=4) as ob,
            tc.tile_pool(name="smsb", bufs=2) as smsb,
            tc.tile_pool(name="ps_dn", bufs=2, space="PSUM") as ps_dn,
            tc.tile_pool(name="ps_up", bufs=2, space="PSUM") as ps_up,
            tc.tile_pool(name="ps_lg", bufs=1, space="PSUM") as ps_lg,
            tc.tile_pool(name="ps_tr", bufs=1, space="PSUM") as ps_tr,
            tc.tile_pool(name="ps_sm", bufs=1, space="PSUM") as ps_sm,
            tc.tile_pool(name="ps_po", bufs=1, space="PSUM") as ps_po,
        ):
            # ---- constants ----
            wdn_sb = const.tile([128, CTILES, 128], BF)
            nc.sync.dma_start(wdn_sb[:], wdn[:])
            wup_sb = const.tile([D, C], BF)
            nc.sync.dma_start(wup_sb[:], wup[:])
            bdn_sb = const.tile([D, 1], F32)
            nc.sync.dma_start(bdn_sb[:], bdn[:])
            bup_sb = const.tile([128, C], F32)
            nc.sync.dma_start(bup_sb[:], bup[:])
            id_sb = const.tile([128, 128], BF)
            nc.sync.dma_start(id_sb[:], ident[:])
            onesb_sb = const.tile([128, 1], BF)
            nc.sync.dma_start(onesb_sb[:], onesb[:])
            ones1_sb = const.tile([1, 128], F32)
            nc.sync.dma_start(ones1_sb[:], ones1[:])

            for b in range(NB):
                # ---- load xT[b] (8 c-tiles) ----
                xsb = xp.tile([128, CTILES, N], BF, tag="xsb")
                for c in range(CTILES):
                    nc.sync.dma_start(xsb[:, c, :], xT[b, c * 128 : (c + 1) * 128, :])

                # ---- down projection: downT[D, N] ----
                dng = dg.tile([128, N], BF, tag="dng")  # gelu(downT), later combT
                for s, w in DOWN_CHUNKS:
                    acc = ps_dn.tile([128, w], F32, tag="dn")
                    for c in range(CTILES):
                        nc.tensor.matmul(
                            acc[:],
                            wdn_sb[:, c, :],
                            xsb[:, c, s : s + w],
                            start=(c == 0),
                            stop=(c == CTILES - 1),
                        )
                    nc.scalar.activation(
                        dng[:, s : s + w],
                        acc[:],
                        mybir.ActivationFunctionType.Gelu,
                        bias=bdn_sb[:],
                        scale=1.0,
                    )

                # ---- attention ----
                # logitsT tiles + exp
                exps = ex.tile([128, TTILES, P], BF, tag="exps")
                for t in range(TTILES):
                    lg = ps_lg.tile([128, P], F32, tag="lg")
                    nc.tensor.matmul(
                        lg[:],
                        dng[:, P + t * 128 : P + (t + 1) * 128],
                        dng[:, 0:P],
                        start=True,
                        stop=True,
                    )
                    nc.scalar.activation(
                        exps[:, t, :],
                        lg[:],
                        mybir.ActivationFunctionType.Exp,
                        scale=SCALE,
                    )

                # transpose token tiles: tok_tr[t] = token_downT_t^T  ([t,D] layout)
                toktr = tt.tile([128, TTILES, 128], BF, tag="toktr")
                for t in range(TTILES):
                    trp = ps_tr.tile([128, 128], BF, tag="tr")
                    nc.tensor.transpose(
                        trp[:], dng[:, P + t * 128 : P + (t + 1) * 128], id_sb[:]
                    )
                    nc.scalar.copy(toktr[:, t, :], trp[:])

                # denom[1, P] = sum_t exp
                den = ps_sm.tile([1, P], F32, tag="sm")
                for t in range(TTILES):
                    nc.tensor.matmul(
                        den[:],
                        onesb_sb[:],
                        exps[:, t, :],
                        start=(t == 0),
                        stop=(t == TTILES - 1),
                    )
                recip = smsb.tile([1, P], F32, tag="recip")
                nc.vector.reciprocal(recip[:], den[:])

                # prompt_outT[D, P] accumulation
                po = ps_po.tile([128, P], F32, tag="po")
                for t in range(TTILES):
                    nc.tensor.matmul(
                        po[:],
                        toktr[:, t, :],
                        exps[:, t, :],
                        start=(t == 0),
                        stop=(t == TTILES - 1),
                    )

                # normalize: combT[:, 0:P] = po * bcast(recip)
                bc = ps_sm.tile([128, P], F32, tag="sm")
                nc.tensor.matmul(bc[:], ones1_sb[:], recip[:], start=True, stop=True)
                bc_sb = smsb.tile([128, P], F32, tag="bcsb")
                nc.scalar.copy(bc_sb[:], bc[:])
                nc.vector.tensor_mul(dng[:, 0:P], po[:], bc_sb[:])

                # ---- up projection + bias + store ----
                for s, rows in UP_TILES:
                    osb = ob.tile([128, C], F32, tag="osb")
                    for h in range(2):
                        up = ps_up.tile([128, 512], F32, tag="up")
                        nc.tensor.matmul(
                            up[:rows, :],
                            dng[:, s : s + rows],
                            wup_sb[:, h * 512 : (h + 1) * 512],
                            start=True,
                            stop=True,
                        )
                        nc.vector.tensor_add(
                            osb[:rows, h * 512 : (h + 1) * 512],
                            up[:rows, :],
                            bup_sb[:rows, h * 512 : (h + 1) * 512],
                        )
                    nc.sync.dma_start(out[b, s : s + rows, :], osb[:rows, :])

    nc.compile()
    return nc


_NC_CACHE = None


def _get_nc():
    global _NC_CACHE
    if _NC_CACHE is None:
        _NC_CACHE = build_nc()
    return _NC_CACHE


def make_in_maps(x, W_down, b_down, W_up, b_up, gate):
    x = np.asarray(x, np.float32)
    W_down = np.asarray(W_down, np.float32)
    b_down = np.asarray(b_down, np.float32)
    W_up = np.asarray(W_up, np.float32)
    b_up = np.asarray(b_up, np.float32)
    gate = float(np.asarray(gate, np.float32))

    bf = ml_dtypes.bfloat16
    xT = np.ascontiguousarray(x.transpose(0, 2, 1)).astype(bf)  # [B, C, N]
    # wdn[p, c, m] = W_down[c*128 + p, m]
    wdn = np.ascontiguousarray(
        W_down.reshape(CTILES, 128, 128).transpose(1, 0, 2)
    ).astype(bf)
    wup = (W_up * gate).astype(bf)  # [D, C]
    bdn = b_down.reshape(D, 1).copy()
    bup = np.tile((b_up * gate).reshape(1, C), (128, 1)).astype(np.float32)
    ident = np.eye(128, dtype=bf)
    onesb = np.ones((128, 1), dtype=bf)
    ones1 = np.ones((1, 128), dtype=np.float32)

    in_maps = []
    for i in range(NCORES):
        in_maps.append(
            {
                "xT": np.ascontiguousarray(xT[i * NB : (i + 1) * NB]),
                "wdn": wdn,
                "wup": wup,
                "bdn": bdn,
                "bup": bup,
                "ident": ident,
                "onesb": onesb,
                "ones1": ones1,
            }
        )
    return in_maps


def kernel(**inputs):
    nc = _get_nc()
    in_maps = make_in_maps(**inputs)
    res = run_bass_kernel_spmd(nc, in_maps, core_ids=list(range(NCORES)))
    out = np.concatenate([res.results[i]["out"] for i in range(NCORES)], axis=0)
    return out.astype(np.float32)


# revision 3
# speedup vs baseline: 1.2062x; 1.2062x over previous
"""Trainium2 Bass kernel for nn_Adapter_CrossNonParam (adapter + prompt/token cross-attention).

Data-parallel over batch: 8 NeuronCores x 4 batches each. Adapter weights are
replicated. All matmuls run in bf16 (fp32 PSUM accumulation); input x is
pre-transposed and cast to bf16 on the host so the device kernel needs no
layout shuffles for the big tensors.

Per-batch device pipeline (everything kept D-on-partition):
  downT[D,N] = W_down^T @ xT            (8 C-tile accumulation, PSUM)
  downT = gelu(downT + b_down)          (ScalarE, erf gelu, fused PSUM->SBUF, bf16)
  logitsT[t,P] = token_downT_t^T @ prompt_downT     (16 tiles)
  expT = exp(SCALE * logitsT)           (no max subtraction; logits are O(5))
  denom[1,P] = ones^T @ expT            (accumulated ones-matmul)
  prompt_outT[D,P] += tok_tr_t^T @ expT_t   (tok_tr = PE-transposed token tiles)
  prompt_outT *= bcast(1/denom)         (PE broadcast + VectorE multiply)
  combT = [prompt_outT | token_downT]   (in-place in the downT buffer)
  up[n,C] = combT_n^T @ W_up ; out = up + b_up   (bias fused into PSUM->SBUF copy)
"""
import numpy as np
import ml_dtypes

import concourse.bass as bass
import concourse.tile as tile
from concourse import bacc, mybir
from concourse.bass_utils import run_bass_kernel_spmd

BF = mybir.dt.bfloat16
F32 = mybir.dt.float32

B, N, C = 32, 2248, 1024
D = 128
P = 200
T = N - P  # 2048
NCORES = 8
NB = B // NCORES  # 4 batches per core
SCALE = float(D) ** -0.5

CTILES = C // 128  # 8
TTILES = T // 128  # 16
DOWN_CHUNKS = [(s, min(512, N - s)) for s in range(0, N, 512)]  # 4x512 + 200
UP_TILES = [(s, min(128, N - s)) for s in range(0, N, 128)]  # 17x128 + 72


def build_nc():
    nc = bacc.Bacc("TRN2", target_bir_lowering=False, debug=False, num_devices=NCORES)

    xT = nc.dram_tensor("xT", [NB, C, N], BF, kind="ExternalInput")
    wdn = nc.dram_tensor("wdn", [128, CTILES, 128], BF, kind="ExternalInput")
    wup = nc.dram_tensor("wup", [D, C], BF, kind="ExternalInput")
    bdn = nc.dram_tensor("bdn", [D, 1], F32, kind="ExternalInput")
    bup = nc.dram_tensor("bup", [128, C], F32, kind="ExternalInput")
    ident = nc.dram_tensor("ident", [128, 128], BF, kind="ExternalInput")
    onesb = nc.dram_tensor("onesb", [128, 1], BF, kind="ExternalInput")
    ones1 = nc.dram_tensor("ones1", [1, 128], F32, kind="ExternalInput")
    out = nc.dram_tensor("out", [NB, N, C], F32, kind="ExternalOutput")

    with tile.TileContext(nc) as tc:
        with (
            tc.tile_pool(name="const", bufs=1) as const,
            tc.tile_pool(name="xp", bufs=2) as xp,
            tc.tile_pool(name="dg", bufs=2) as dg,
            tc.tile_pool(name="ex", bufs=2) as ex,
            tc.tile_pool(name="tt", bufs=2) as tt,
            tc.tile_pool(name="ob", bufs=4) as ob,
            tc.tile_pool(name="smsb", bufs=2) as smsb,
            tc.tile_pool(name="ps_dn", bufs=2, space="PSUM") as ps_dn,
            tc.tile_pool(name="ps_up", bufs=2, space="PSUM") as ps_up,
            tc.tile_pool(name="ps_lg", bufs=1, space="PSUM") as ps_lg,
            tc.tile_pool(name="ps_tr", bufs=1, space="PSUM") as ps_tr,
            tc.tile_pool(name="ps_sm", bufs=1, space="PSUM") as ps_sm,
            tc.tile_pool(name="ps_po", bufs=1, space="PSUM") as ps_po,
        ):
            # ---- constants ----
            wdn_sb = const.tile([128, CTILES, 128], BF)
            nc.sync.dma_start(wdn_sb[:], wdn[:])
            wup_sb = const.tile([D, C], BF)
            nc.sync.dma_start(wup_sb[:], wup[:])
            bdn_sb = const.tile([D, 1], F32)
            nc.sync.dma_start(bdn_sb[:], bdn[:])
            bup_sb = const.tile([128, C], F32)
            nc.sync.dma_start(bup_sb[:], bup[:])
            id_sb = const.tile([128, 128], BF)
            nc.sync.dma_start(id_sb[:], ident[:])
            onesb_sb = const.tile([128, 1], BF)
            nc.sync.dma_start(onesb_sb[:], onesb[:])
            ones1_sb = const.tile([1, 128], F32)
            nc.sync.dma_start(ones1_sb[:], ones1[:])

            for b in range(NB):
                # ---- load xT[b] (8 c-tiles, two 2.25MB DMAs) ----
                xsb = xp.tile([128, CTILES, N], BF, tag="xsb")
                for h in range(2):
                    src = xT[b, h * 512 : (h + 1) * 512, :].rearrange(
                        "(a p) n -> p a n", p=128
                    )
                    nc.sync.dma_start(xsb[:, h * 4 : (h + 1) * 4, :], src)

                # ---- down projection: downT[D, N] ----
                dng = dg.tile([128, N], BF, tag="dng")  # gelu(downT), later combT
                for s, w in DOWN_CHUNKS:
                    acc = ps_dn.tile([128, w], F32, tag="dn")
                    for c in range(CTILES):
                        nc.tensor.matmul(
                            acc[:],
                            wdn_sb[:, c, :],
                            xsb[:, c, s : s + w],
                            start=(c == 0),
                            stop=(c == CTILES - 1),
                        )
                    nc.scalar.activation(
                        dng[:, s : s + w],
                        acc[:],
                        mybir.ActivationFunctionType.Gelu,
                        bias=bdn_sb[:],
                        scale=1.0,
                    )

                # ---- attention ----
                # logitsT tiles + exp
                exps = ex.tile([128, TTILES, P], BF, tag="exps")
                for t in range(TTILES):
                    lg = ps_lg.tile([128, P], F32, tag="lg")
                    nc.tensor.matmul(
                        lg[:],
                        dng[:, P + t * 128 : P + (t + 1) * 128],
                        dng[:, 0:P],
                        start=True,
                        stop=True,
                    )
                    nc.scalar.activation(
                        exps[:, t, :],
                        lg[:],
                        mybir.ActivationFunctionType.Exp,
                        scale=SCALE,
                    )

                # transpose token tiles: tok_tr[t] = token_downT_t^T  ([t,D] layout)
                toktr = tt.tile([128, TTILES, 128], BF, tag="toktr")
                for t in range(TTILES):
                    trp = ps_tr.tile([128, 128], BF, tag="tr")
                    nc.tensor.transpose(
                        trp[:], dng[:, P + t * 128 : P + (t + 1) * 128], id_sb[:]
                    )
                    nc.scalar.copy(toktr[:, t, :], trp[:])

                # denom[1, P] = sum_t exp
                den = ps_sm.tile([1, P], F32, tag="sm")
                for t in range(TTILES):
                    nc.tensor.matmul(
                        den[:],
                        onesb_sb[:],
                        exps[:, t, :],
                        start=(t == 0),
                        stop=(t == TTILES - 1),
                    )
                recip = smsb.tile([1, P], F32, tag="recip")
                nc.vector.reciprocal(recip[:], den[:])

                # prompt_outT[D, P] accumulation
                po = ps_po.tile([128, P], F32, tag="po")
                for t in range(TTILES):
                    nc.tensor.matmul(
                        po[:],
                        toktr[:, t, :],
                        exps[:, t, :],
                        start=(t == 0),
                        stop=(t == TTILES - 1),
                    )

                # ---- up projection helper: one m-tile into osb[:, jj, :] ----
                def up_mtile(mi, osb, jj):
                    s, rows = UP_TILES[mi]
                    for h in range(2):
                        up = ps_up.tile([128, 512], F32, tag="up")
                        nc.tensor.matmul(
                            up[:rows, :],
                            dng[:, s : s + rows],
                            wup_sb[:, h * 512 : (h + 1) * 512],
                            start=True,
                            stop=True,
                        )
                        nc.vector.tensor_add(
                            osb[:rows, jj, h * 512 : (h + 1) * 512],
                            up[:rows, :],
                            bup_sb[:rows, h * 512 : (h + 1) * 512],
                        )

                # token-row pairs (independent of attention) first: overlaps
                # the attention serial chain. pair j covers m-tiles (2j, 2j+1).
                for j in range(1, 8):
                    osb = ob.tile([128, 2, C], F32, tag="osb")
                    up_mtile(2 * j, osb, 0)
                    up_mtile(2 * j + 1, osb, 1)
                    dst = out[b, 256 * j : 256 * (j + 1), :].rearrange(
                        "(a p) c -> p a c", p=128
                    )
                    nc.sync.dma_start(dst, osb[:])
                for mi in (16, 17):
                    s, rows = UP_TILES[mi]
                    osb = ob.tile([128, 2, C], F32, tag="osb")
                    up_mtile(mi, osb, 0)
                    nc.sync.dma_start(out[b, s : s + rows, :], osb[:rows, 0, :])

                # normalize: combT[:, 0:P] = po * bcast(recip)
                bc = ps_sm.tile([128, P], F32, tag="sm")
                nc.tensor.matmul(bc[:], ones1_sb[:], recip[:], start=True, stop=True)
                bc_sb = smsb.tile([128, P], F32, tag="bcsb")
                nc.scalar.copy(bc_sb[:], bc[:])
                nc.vector.tensor_mul(dng[:, 0:P], po[:], bc_sb[:])

                # prompt-row pair (m-tiles 0, 1) after normalization
                osb = ob.tile([128, 2, C], F32, tag="osb")
                up_mtile(0, osb, 0)
                up_mtile(1, osb, 1)
                dst = out[b, 0:256, :].rearrange("(a p) c -> p a c", p=128)
                nc.sync.dma_start(dst, osb[:])

    nc.compile()
    return nc


_NC_CACHE = None


def _get_nc():
    global _NC_CACHE
    if _NC_CACHE is None:
        _NC_CACHE = build_nc()
    return _NC_CACHE


def make_in_maps(x, W_down, b_down, W_up, b_up, gate):
    x = np.asarray(x, np.float32)
    W_down = np.asarray(W_down, np.float32)
    b_down = np.asarray(b_down, np.float32)
    W_up = np.asarray(W_up, np.float32)
    b_up = np.asarray(b_up, np.float32)
    gate = float(np.asarray(gate, np.float32))

    bf = ml_dtypes.bfloat16
    xT = np.ascontiguousarray(x.transpose(0, 2, 1)).astype(bf)  # [B, C, N]
    # wdn[p, c, m] = W_down[c*128 + p, m]
    wdn = np.ascontiguousarray(
        W_down.reshape(CTILES, 128, 128).transpose(1, 0, 2)
    ).astype(bf)
    wup = (W_up * gate).astype(bf)  # [D, C]
    bdn = b_down.reshape(D, 1).copy()
    bup = np.tile((b_up * gate).reshape(1, C), (128, 1)).astype(np.float32)
    ident = np.eye(128, dtype=bf)
    onesb = np.ones((128, 1), dtype=bf)
    ones1 = np.ones((1, 128), dtype=np.float32)

    in_maps = []
    for i in range(NCORES):
        in_maps.append(
            {
                "xT": np.ascontiguousarray(xT[i * NB : (i + 1) * NB]),
                "wdn": wdn,
                "wup": wup,
                "bdn": bdn,
                "bup": bup,
                "ident": ident,
                "onesb": onesb,
                "ones1": ones1,
            }
        )
    return in_maps


def kernel(**inputs):
    nc = _get_nc()
    in_maps = make_in_maps(**inputs)
    res = run_bass_kernel_spmd(nc, in_maps, core_ids=list(range(NCORES)))
    out = np.concatenate([res.results[i]["out"] for i in range(NCORES)], axis=0)
    return out.astype(np.float32)


# revision 6
# speedup vs baseline: 1.2348x; 1.0238x over previous
"""Trainium2 Bass kernel for nn_Adapter_CrossNonParam (adapter + prompt/token cross-attention).

Data-parallel over batch: 8 NeuronCores x 4 batches each. Adapter weights are
replicated. All matmuls run in bf16 (fp32 PSUM accumulation); input x is
pre-transposed and cast to bf16 on the host so the device kernel needs no
layout shuffles for the big tensors.

Per-batch device pipeline (everything kept D-on-partition):
  downT[D,N] = W_down^T @ xT            (8 C-tile accumulation, PSUM)
  downT = gelu(downT + b_down)          (ScalarE, erf gelu, fused PSUM->SBUF, bf16)
  logitsT[t,P] = token_downT_t^T @ prompt_downT     (16 tiles)
  expT = exp(SCALE * logitsT)           (no max subtraction; logits are O(5))
  denom[1,P] = ones^T @ expT            (accumulated ones-matmul)
  prompt_outT[D,P] += tok_tr_t^T @ expT_t   (tok_tr = PE-transposed token tiles)
  prompt_outT *= bcast(1/denom)         (PE broadcast + VectorE multiply)
  combT = [prompt_outT | token_downT]   (in-place in the downT buffer)
  up[n,C] = combT_n^T @ W_up ; out = up + b_up   (bias fused into PSUM->SBUF copy)
"""
import numpy as np
import ml_dtypes

import concourse.bass as bass
import concourse.tile as tile
from concourse import bacc, mybir
from concourse.bass_utils import run_bass_kernel_spmd

BF = mybir.dt.bfloat16
F32 = mybir.dt.float32

B, N, C = 32, 2248, 1024
D = 128
P = 200
T = N - P  # 2048
NCORES = 8
NB = B // NCORES  # 4 batches per core
SCALE = float(D) ** -0.5

CTILES = C // 128  # 8
TTILES = T // 128  # 16
DOWN_CHUNKS = [(s, min(512, N - s)) for s in range(0, N, 512)]  # 4x512 + 200
UP_TILES = [(s, min(128, N - s)) for s in range(0, N, 128)]  # 17x128 + 72


def build_nc():
    nc = bacc.Bacc("TRN2", target_bir_lowering=False, debug=False, num_devices=NCORES)

    xT = nc.dram_tensor("xT", [NB, C, N], BF, kind="ExternalInput")
    wdn = nc.dram_tensor("wdn", [128, CTILES, 128], BF, kind="ExternalInput")
    wup = nc.dram_tensor("wup", [D, C], BF, kind="ExternalInput")
    bdn = nc.dram_tensor("bdn", [D, 1], F32, kind="ExternalInput")
    bup = nc.dram_tensor("bup", [128, C], F32, kind="ExternalInput")
    ident = nc.dram_tensor("ident", [128, 128], BF, kind="ExternalInput")
    onesb = nc.dram_tensor("onesb", [128, 1], BF, kind="ExternalInput")
    ones1 = nc.dram_tensor("ones1", [1, 128], F32, kind="ExternalInput")
    out = nc.dram_tensor("out", [NB, N, C], F32, kind="ExternalOutput")

    with tile.TileContext(nc) as tc:
        with (
            tc.tile_pool(name="const", bufs=1) as const,
            tc.tile_pool(name="xp", bufs=2) as xp,
            tc.tile_pool(name="dg", bufs=2) as dg,
            tc.tile_pool(name="ex", bufs=2) as ex,
            tc.tile_pool(name="tt", bufs=2) as tt,
            tc.tile_pool(name="ob", bufs=4) as ob,
            tc.tile_pool(name="smsb", bufs=2) as smsb,
            tc.tile_pool(name="ps_dn", bufs=2, space="PSUM") as ps_dn,
            tc.tile_pool(name="ps_up", bufs=2, space="PSUM") as ps_up,
            tc.tile_pool(name="ps_lg", bufs=1, space="PSUM") as ps_lg,
            tc.tile_pool(name="ps_tr", bufs=1, space="PSUM") as ps_tr,
            tc.tile_pool(name="ps_sm", bufs=1, space="PSUM") as ps_sm,
            tc.tile_pool(name="ps_po", bufs=1, space="PSUM") as ps_po,
        ):
            xsb_tiles = {}

            def load_x(b, split):
                """Issue DMA loads for batch b's xT. split=True → 8 per-c-tile
                DMAs (fine-grained deps, fast ramp); else 2 large DMAs."""
                xsb = xp.tile([128, CTILES, N], BF, tag="xsb")
                xsb_tiles[b] = xsb
                if split:
                    for c in range(CTILES):
                        nc.sync.dma_start(
                            xsb[:, c, :], xT[b, c * 128 : (c + 1) * 128, :]
                        )
                else:
                    for h in range(2):
                        src = xT[b, h * 512 : (h + 1) * 512, :].rearrange(
                            "(a p) n -> p a n", p=128
                        )
                        nc.sync.dma_start(xsb[:, h * 4 : (h + 1) * 4, :], src)

            # batch 0 load first so the first down-matmuls ramp ASAP
            load_x(0, split=True)

            # ---- constants ----
            wdn_sb = const.tile([128, CTILES, 128], BF)
            nc.sync.dma_start(wdn_sb[:], wdn[:])
            wup_sb = const.tile([D, C], BF)
            nc.sync.dma_start(wup_sb[:], wup[:])
            bdn_sb = const.tile([D, 1], F32)
            nc.sync.dma_start(bdn_sb[:], bdn[:])
            bup_sb = const.tile([128, C], F32)
            nc.sync.dma_start(bup_sb[:], bup[:])
            id_sb = const.tile([128, 128], BF)
            nc.sync.dma_start(id_sb[:], ident[:])
            onesb_sb = const.tile([128, 1], BF)
            nc.sync.dma_start(onesb_sb[:], onesb[:])
            ones1_sb = const.tile([1, 128], F32)
            nc.sync.dma_start(ones1_sb[:], ones1[:])

            for b in range(NB):
                xsb = xsb_tiles[b]

                # ---- down projection: downT[D, N] ----
                dng = dg.tile([128, N], BF, tag="dng")  # gelu(downT), later combT
                for s, w in DOWN_CHUNKS:
                    acc = ps_dn.tile([128, w], F32, tag="dn")
                    for c in range(CTILES):
                        nc.tensor.matmul(
                            acc[:],
                            wdn_sb[:, c, :],
                            xsb[:, c, s : s + w],
                            start=(c == 0),
                            stop=(c == CTILES - 1),
                        )
                    nc.scalar.activation(
                        dng[:, s : s + w],
                        acc[:],
                        mybir.ActivationFunctionType.Gelu,
                        bias=bdn_sb[:],
                        scale=1.0,
                    )

                # prefetch next batch's x right after this batch's down-matmuls
                # (before the out-stores land on the in-order sync DMA ring)
                if b + 1 < NB:
                    load_x(b + 1, split=False)

                # ---- attention ----
                # logitsT tiles + exp
                exps = ex.tile([128, TTILES, P], BF, tag="exps")
                for t in range(TTILES):
                    lg = ps_lg.tile([128, P], F32, tag="lg")
                    nc.tensor.matmul(
                        lg[:],
                        dng[:, P + t * 128 : P + (t + 1) * 128],
                        dng[:, 0:P],
                        start=True,
                        stop=True,
                    )
                    nc.scalar.activation(
                        exps[:, t, :],
                        lg[:],
                        mybir.ActivationFunctionType.Exp,
                        scale=SCALE,
                    )

                # transpose token tiles: tok_tr[t] = token_downT_t^T  ([t,D] layout)
                toktr = tt.tile([128, TTILES, 128], BF, tag="toktr")
                for t in range(TTILES):
                    trp = ps_tr.tile([128, 128], BF, tag="tr")
                    nc.tensor.transpose(
                        trp[:], dng[:, P + t * 128 : P + (t + 1) * 128], id_sb[:]
                    )
                    nc.scalar.copy(toktr[:, t, :], trp[:])

                # denom[1, P] = sum_t exp
                den = ps_sm.tile([1, P], F32, tag="sm")
                for t in range(TTILES):
                    nc.tensor.matmul(
                        den[:],
                        onesb_sb[:],
                        exps[:, t, :],
                        start=(t == 0),
                        stop=(t == TTILES - 1),
                    )
                recip = smsb.tile([1, P], F32, tag="recip")
                nc.vector.reciprocal(recip[:], den[:])

                # prompt_outT[D, P] accumulation
                po = ps_po.tile([128, P], F32, tag="po")
                for t in range(TTILES):
                    nc.tensor.matmul(
                        po[:],
                        toktr[:, t, :],
                        exps[:, t, :],
                        start=(t == 0),
                        stop=(t == TTILES - 1),
                    )

                # ---- up projection helper: one m-tile into osb[:, jj, :] ----
                def up_mtile(mi, osb, jj):
                    s, rows = UP_TILES[mi]
                    for h in range(2):
                        up = ps_up.tile([128, 512], F32, tag="up")
                        nc.tensor.matmul(
                            up[:rows, :],
                            dng[:, s : s + rows],
                            wup_sb[:, h * 512 : (h + 1) * 512],
                            start=True,
                            stop=True,
                        )
                        nc.vector.tensor_add(
                            osb[:rows, jj, h * 512 : (h + 1) * 512],
                            up[:rows, :],
                            bup_sb[:rows, h * 512 : (h + 1) * 512],
                        )

                # token-row pairs (independent of attention) first: overlaps
                # the attention serial chain. pair j covers m-tiles (2j, 2j+1).
                for j in range(1, 8):
                    osb = ob.tile([128, 2, C], F32, tag="osb")
                    up_mtile(2 * j, osb, 0)
                    up_mtile(2 * j + 1, osb, 1)
                    dst = out[b, 256 * j : 256 * (j + 1), :].rearrange(
                        "(a p) c -> p a c", p=128
                    )
                    nc.sync.dma_start(dst, osb[:])
                for mi in (16, 17):
                    s, rows = UP_TILES[mi]
                    osb = ob.tile([128, 2, C], F32, tag="osb")
                    up_mtile(mi, osb, 0)
                    nc.sync.dma_start(out[b, s : s + rows, :], osb[:rows, 0, :])

                # normalize: combT[:, 0:P] = po * bcast(recip)
                bc = ps_sm.tile([128, P], F32, tag="sm")
                nc.tensor.matmul(bc[:], ones1_sb[:], recip[:], start=True, stop=True)
                bc_sb = smsb.tile([128, P], F32, tag="bcsb")
                nc.scalar.copy(bc_sb[:], bc[:])
                nc.vector.tensor_mul(dng[:, 0:P], po[:], bc_sb[:])

                # prompt-row pair (m-tiles 0, 1) after normalization
                osb = ob.tile([128, 2, C], F32, tag="osb")
                up_mtile(0, osb, 0)
                up_mtile(1, osb, 1)
                dst = out[b, 0:256, :].rearrange("(a p) c -> p a c", p=128)
                nc.sync.dma_start(dst, osb[:])

    nc.compile()
    return nc


_NC_CACHE = None


def _get_nc():
    global _NC_CACHE
    if _NC_CACHE is None:
        _NC_CACHE = build_nc()
    return _NC_CACHE


def make_in_maps(x, W_down, b_down, W_up, b_up, gate):
    x = np.asarray(x, np.float32)
    W_down = np.asarray(W_down, np.float32)
    b_down = np.asarray(b_down, np.float32)
    W_up = np.asarray(W_up, np.float32)
    b_up = np.asarray(b_up, np.float32)
    gate = float(np.asarray(gate, np.float32))

    bf = ml_dtypes.bfloat16
    xT = np.ascontiguousarray(x.transpose(0, 2, 1)).astype(bf)  # [B, C, N]
    # wdn[p, c, m] = W_down[c*128 + p, m]
    wdn = np.ascontiguousarray(
        W_down.reshape(CTILES, 128, 128).transpose(1, 0, 2)
    ).astype(bf)
    wup = (W_up * gate).astype(bf)  # [D, C]
    bdn = b_down.reshape(D, 1).copy()
    bup = np.tile((b_up * gate).reshape(1, C), (128, 1)).astype(np.float32)
    ident = np.eye(128, dtype=bf)
    onesb = np.ones((128, 1), dtype=bf)
    ones1 = np.ones((1, 128), dtype=np.float32)

    in_maps = []
    for i in range(NCORES):
        in_maps.append(
            {
                "xT": np.ascontiguousarray(xT[i * NB : (i + 1) * NB]),
                "wdn": wdn,
                "wup": wup,
                "bdn": bdn,
                "bup": bup,
                "ident": ident,
                "onesb": onesb,
                "ones1": ones1,
            }
        )
    return in_maps


def kernel(**inputs):
    nc = _get_nc()
    in_maps = make_in_maps(**inputs)
    res = run_bass_kernel_spmd(nc, in_maps, core_ids=list(range(NCORES)))
    out = np.concatenate([res.results[i]["out"] for i in range(NCORES)], axis=0)
    return out.astype(np.float32)
